# revision 18
# baseline (speedup 1.0000x reference)
"""Trainium2 Bass kernel for nn_Attention (B=4, N=2048, D=1024, H=16, Hd=64).

Sharding: 8 cores = 4 batches x 2 head-groups. Core c handles batch c//2 and
heads [ (c%2)*8, (c%2)*8+8 ).  Each core computes qkv projections for its
heads, attention, and a partial output projection (contraction over its 512
head-dims of W_proj). Host sums the two partials per batch and adds b_proj.

Per-core kernel (all matmuls bf16 with fp32 PSUM accumulation):
  - qkT[f, t]  = sum_d Wqk[d, f] * xT[d, t]     (Q^T/K^T per head, [64, 2048])
  - v[t, f]    = sum_d xT[d, t] * Wv[d, f]       ([2048, 512], keys-major)
  - per head pair (2 heads packed in PE row/col groups):
      S^T[k, q] = sum_d K^T[d, k] Q^T[d, q]      (keys on partitions)
      E = exp(S^T / 8)   (ScalarE, bf16 out)
      U^T[hd, q] += sum_k V[k, hd] E[k, q]       (PSUM accumulate over key tiles)
      Eacc += E (VectorE);  sums = partition_all_reduce(Eacc)  (GpSimd)
      Uhat = U^T * (1/sums)                      (normalize during PSUM drain)
  - y[q, e] = sum_hd Uhat[hd, q] Wp[hd, e]       (partial; host adds pair+bias)
"""

import os
import sys
import types

import numpy as np

# --- environment bootstrap (grading env == dev env: axon-tunneled trn2) ----
for _p in ("/opt/trn_rl_repo", "/root/.axon_site/_ro/trn_rl_repo"):
    if _p not in sys.path and os.path.isdir(_p):
        sys.path.append(_p)

import ml_dtypes  # noqa: E402

BF16 = ml_dtypes.bfloat16


def _install_ntff_shim():
    """antenv.axon_hooks is missing on this image; provide it and register the
    ctypes NTFF hook so trace=True can report HW exec time."""
    if "antenv.axon_hooks" in sys.modules:
        return
    mod = types.ModuleType("antenv.axon_hooks")
    mod._hook = None
    mod.set_axon_ntff_profile_hook = lambda h: setattr(mod, "_hook", h)
    mod.get_axon_ntff_profile_hook = lambda: mod._hook
    sys.modules["antenv.axon_hooks"] = mod
    try:
        import antenv

        antenv.axon_hooks = mod
    except ImportError:
        pass
    try:
        from trn_agent_boot.trn_boot import _ntff_profile_via_ctypes

        hook = _ntff_profile_via_ctypes("/opt/axon/libaxon_pjrt.so")
        if hook is not None:
            mod.set_axon_ntff_profile_hook(hook)
    except Exception:
        pass


_install_ntff_shim()

import concourse.bacc as bacc  # noqa: E402
import concourse.bass as bass  # noqa: E402
import concourse.tile as tile  # noqa: E402
import concourse.bass_isa as bass_isa  # noqa: E402
from concourse import mybir  # noqa: E402
import concourse.bass_utils as bass_utils  # noqa: E402

# no S3 in the container; keep NTFF artifacts local
bass_utils.upload_artifacts = lambda tmpdir: tmpdir

F32 = mybir.dt.float32
BF = mybir.dt.bfloat16
EXP = mybir.ActivationFunctionType.Exp

N_CORES = 8
NT = 2048  # tokens
D = 1024  # d_model
NH_LOC = 8  # heads per core
HD = 64  # head dim
SCALE = HD**-0.5


def _body(tc: "tile.TileContext", ctx, y, xT, wqk, wv, wp):
    nc = tc.nc

    wpool = ctx.enter_context(tc.tile_pool(name="wpool", bufs=1))
    qkpool = ctx.enter_context(tc.tile_pool(name="qkpool", bufs=1))
    vpool = ctx.enter_context(tc.tile_pool(name="vpool", bufs=1))
    upool = ctx.enter_context(tc.tile_pool(name="upool", bufs=1))
    epool = ctx.enter_context(tc.tile_pool(name="epool", bufs=5))
    eaccpool = ctx.enter_context(tc.tile_pool(name="eaccpool", bufs=3))
    spool = ctx.enter_context(tc.tile_pool(name="spool", bufs=1))
    rpool = ctx.enter_context(tc.tile_pool(name="rpool", bufs=1))
    opool = ctx.enter_context(tc.tile_pool(name="opool", bufs=2))
    psb = ctx.enter_context(tc.tile_pool(name="psb", bufs=2, space="PSUM"))
    psu = ctx.enter_context(tc.tile_pool(name="psu", bufs=1, space="PSUM"))

    # ---- persistent SBUF tensors -----------------------------------------
    xT_sb = []
    for i in range(8):
        t = wpool.tile([128, NT], BF, tag=f"xT{i}", name=f"xT{i}")
        nc.sync.dma_start(out=t, in_=xT[i * 128 : (i + 1) * 128, :])
        xT_sb.append(t)
    wqk_sb = []
    for i in range(8):
        t = wpool.tile([128, 1024], BF, tag=f"wqk{i}", name=f"wqk{i}")
        nc.sync.dma_start(out=t, in_=wqk[i * 128 : (i + 1) * 128, :])
        wqk_sb.append(t)
    wv_sb = []
    for i in range(8):
        t = wpool.tile([128, 512], BF, tag=f"wv{i}", name=f"wv{i}")
        nc.sync.dma_start(out=t, in_=wv[i * 128 : (i + 1) * 128, :])
        wv_sb.append(t)
    wp_sb = []
    for i in range(4):
        t = wpool.tile([128, 1024], BF, tag=f"wp{i}", name=f"wp{i}")
        nc.sync.dma_start(out=t, in_=wp[i * 128 : (i + 1) * 128, :])
        wp_sb.append(t)

    qkT = [qkpool.tile([128, NT], BF, tag=f"qkT{f}", name=f"qkT{f}") for f in range(8)]
    v_sb = [vpool.tile([128, 512], BF, tag=f"v{t}", name=f"v{t}") for t in range(16)]
    uhat = [upool.tile([128, NT], BF, tag=f"uh{p}", name=f"uh{p}") for p in range(4)]
    ones64 = wpool.tile([128, 64], BF, tag="ones64", name="ones64")
    nc.vector.memset(ones64, 1.0)

    # ---- qkv projections --------------------------------------------------
    def qk_unit(f):
        # qkT[f][ff, t] = sum_d wqk[d, f*128+ff] * xT[d, t]
        for ts in range(2):
            ps = psb.tile([128, 1024], F32, tag="psb", name=f"qk_ps{f}_{ts}")
            for d in range(8):
                for s in range(2):
                    nc.tensor.matmul(
                        ps[:, s * 512 : (s + 1) * 512],
                        wqk_sb[d][:, f * 128 : (f + 1) * 128],
                        xT_sb[d][:, ts * 1024 + s * 512 : ts * 1024 + (s + 1) * 512],
                        start=(d == 0),
                        stop=(d == 7),
                    )
            nc.vector.tensor_copy(out=qkT[f][:, ts * 1024 : (ts + 1) * 1024], in_=ps[:])

    def v_unit(t):
        # v[t*128+tt, f] = sum_d xT[d, t*128+tt] * wv[d, f]; psum shares psu pool
        ps = psu.tile([128, 512], F32, tag="ut", name=f"v_ps{t}")
        for d in range(8):
            nc.tensor.matmul(
                ps[:, :],
                xT_sb[d][:, t * 128 : (t + 1) * 128],
                wv_sb[d][:, :],
                start=(d == 0),
                stop=(d == 7),
            )
        nc.vector.tensor_copy(out=v_sb[t], in_=ps[:])

    # ---- attention for one pair of heads (A=2p at rows 0:64, B at 64:128) -
    def attention_pair(p):
        A, B = 2 * p, 2 * p + 1
        qA = qkT[p][0:64, :]
        qB = qkT[p][64:128, :]
        kA = qkT[4 + p][0:64, :]
        kB = qkT[4 + p][64:128, :]
        ut = psu.tile([128, NT], F32, tag="ut", name=f"ut{p}")
        eaccA = eaccpool.tile([128, NT], BF, tag="eacc", name=f"eaccA{p}")
        eaccB = eaccpool.tile([128, NT], BF, tag="eacc", name=f"eaccB{p}")
        # Software-pipelined kt loop: the PV matmuls for key-tile kt-1 (inputs
        # long ready) are emitted between the QK score groups so the in-order
        # PE queue never stalls on the ACT exp stagger.
        eAs, eBs = [], []

        def pv(ktpv, half, s):
            e, head, r0 = (eAs[ktpv], A, 0) if half == 0 else (eBs[ktpv], B, 64)
            ssl = slice(s * 512, (s + 1) * 512)
            nc.tensor.matmul(
                ut[r0 : r0 + 64, ssl],
                v_sb[ktpv][:, head * 64 : (head + 1) * 64],
                e[:, ssl],
                start=(ktpv == 0),
                stop=(ktpv == 15),
            )

        def eacc_step(ktpv):
            if ktpv == 0:
                nc.vector.tensor_copy(out=eaccA, in_=eAs[0])
                nc.vector.tensor_copy(out=eaccB, in_=eBs[0])
            else:
                nc.vector.tensor_add(out=eaccA, in0=eaccA, in1=eAs[ktpv])
                nc.vector.tensor_add(out=eaccB, in0=eaccB, in1=eBs[ktpv])

        for kt in range(16):
            ksl = slice(kt * 128, (kt + 1) * 128)
            eA = epool.tile([128, NT], BF, tag="e", name=f"eA{p}_{kt}")
            eB = epool.tile([128, NT], BF, tag="e", name=f"eB{p}_{kt}")
            eAs.append(eA)
            eBs.append(eB)
            for qh in range(2):
                qsl = slice(qh * 1024, (qh + 1) * 1024)
                stA = psb.tile([128, 1024], F32, tag="psb", name=f"stA{p}_{kt}_{qh}")
                for s in range(2):
                    q0 = qh * 1024 + s * 512
                    nc.tensor.matmul(
                        stA[:, s * 512 : (s + 1) * 512], kA[:, ksl], qA[:, q0 : q0 + 512],
                        start=True, stop=True,
                    )
                nc.scalar.activation(out=eA[:, qsl], in_=stA[:], func=EXP, scale=SCALE)
                if kt > 0:
                    pv(kt - 1, 0, 2 * qh)
                    pv(kt - 1, 0, 2 * qh + 1)
                stB = psb.tile([128, 1024], F32, tag="psb", name=f"stB{p}_{kt}_{qh}")
                for s in range(2):
                    q0 = qh * 1024 + s * 512
                    nc.tensor.matmul(
                        stB[:, s * 512 : (s + 1) * 512], kB[:, ksl], qB[:, q0 : q0 + 512],
                        start=True, stop=True,
                    )
                nc.scalar.activation(out=eB[:, qsl], in_=stB[:], func=EXP, scale=SCALE)
                if kt > 0:
                    pv(kt - 1, 1, 2 * qh)
                    pv(kt - 1, 1, 2 * qh + 1)
            if kt > 0:
                eacc_step(kt - 1)
        # drain the pipelined tail (kt = 15)
        for s in range(4):
            pv(15, 0, s)
            pv(15, 1, s)
        eacc_step(15)
        # drain U^T (unnormalized) to SBUF right away so the PSUM accumulator
        # frees for the next pair; normalization happens off the critical path.
        for qh in range(2):
            qsl = slice(qh * 1024, (qh + 1) * 1024)
            nc.vector.tensor_copy(out=uhat[p][:, qsl], in_=ut[:, qsl])
        # softmax denominators: sums over the 128 key partitions via a
        # ones-matmul (PE, broadcast into rows; A -> 0:64, B -> 64:128) into
        # psb tiles (so they don't serialize behind the U^T drain), then
        # reciprocal on a partition-spread [128, 16] view (DVE divide is
        # ~6 cycles/elem, so never run it on a 2048-wide row), then broadcast
        # the reciprocal row to all partitions (GpSimd partition_broadcast).
        sums = spool.tile([128, NT], F32, tag="sums", name=f"sums{p}")
        for qh in range(2):
            qsl = slice(qh * 1024, (qh + 1) * 1024)
            sums_ps = psb.tile([128, 1024], F32, tag="psb", name=f"sums_ps{p}_{qh}")
            for s in range(2):
                ssl = slice(qh * 1024 + s * 512, qh * 1024 + (s + 1) * 512)
                osl = slice(s * 512, (s + 1) * 512)
                nc.tensor.matmul(sums_ps[0:64, osl], ones64[:], eaccA[:, ssl], start=True, stop=True)
                nc.tensor.matmul(sums_ps[64:128, osl], ones64[:], eaccB[:, ssl], start=True, stop=True)
            nc.vector.tensor_copy(out=sums[:, qsl], in_=sums_ps[:])
        for half in (0, 1):
            r0 = half * 64
            rsp = spool.tile([128, 16], F32, tag="rsp", name=f"rsp{p}_{half}")
            row = sums[r0 : r0 + 1, :].rearrange("p (a b) -> p a b", a=128)
            nc.gpsimd.dma_start(out=rsp[:], in_=row)
            rspr = spool.tile([128, 16], F32, tag="rspr", name=f"rspr{p}_{half}")
            nc.vector.reciprocal(out=rspr[:], in_=rsp[:])
            rrow = spool.tile([1, NT], F32, tag="rrow", bufs=1, name=f"rrow{p}_{half}")
            nc.gpsimd.dma_start(
                out=rrow[0:1, :].rearrange("p (a b) -> p a b", a=128), in_=rspr[:]
            )
            rec = rpool.tile([128, NT], F32, tag=f"rec{half}", name=f"rec{p}_{half}")
            nc.gpsimd.partition_broadcast(out_ap=rec[:, :], in_ap=rrow[0:1, :])
            # normalize this half in place (bf16 * f32 -> bf16)
            for qh in range(2):
                qsl = slice(qh * 1024, (qh + 1) * 1024)
                nc.vector.tensor_mul(
                    uhat[p][r0 : r0 + 64, qsl], uhat[p][r0 : r0 + 64, qsl], rec[r0 : r0 + 64, qsl]
                )

    # ---- output projection (partial over local 512 head dims) ------------
    def proj_unit(qt):
        pj = psb.tile([128, 1024], F32, tag="psb", name=f"pj{qt}")
        for es in range(2):
            for c in range(4):
                nc.tensor.matmul(
                    pj[:, es * 512 : (es + 1) * 512],
                    uhat[c][:, qt * 128 : (qt + 1) * 128],
                    wp_sb[c][:, es * 512 : (es + 1) * 512],
                    start=(c == 0),
                    stop=(c == 3),
                )
        ot = opool.tile([128, 1024], F32, tag="out", name=f"ot{qt}")
        nc.vector.tensor_copy(out=ot, in_=pj[:])
        nc.sync.dma_start(out=y[qt * 128 : (qt + 1) * 128, :], in_=ot)

    # ---- schedule ---------------------------------------------------------
    qk_unit(0)
    qk_unit(4)
    for t in range(16):
        v_unit(t)
    qk_unit(1)
    qk_unit(5)
    qk_unit(2)
    qk_unit(6)
    qk_unit(3)
    qk_unit(7)
    for p in range(4):
        attention_pair(p)
    for qt in range(16):
        proj_unit(qt)


_NC_CACHE = {}


def _build_nc():
    if "nc" in _NC_CACHE:
        return _NC_CACHE["nc"]
    nc = bacc.Bacc("TRN2", target_bir_lowering=False, debug=False, num_devices=N_CORES)
    xT = nc.dram_tensor("xT", [D, NT], BF, kind="ExternalInput").ap()
    wqk = nc.dram_tensor("wqk", [D, 1024], BF, kind="ExternalInput").ap()
    wv = nc.dram_tensor("wv", [D, 512], BF, kind="ExternalInput").ap()
    wp = nc.dram_tensor("wp", [512, 1024], BF, kind="ExternalInput").ap()
    y = nc.dram_tensor("y", [NT, 1024], F32, kind="ExternalOutput").ap()
    from contextlib import ExitStack

    with tile.TileContext(nc) as tc, ExitStack() as ctx:
        _body(tc, ctx, y, xT, wqk, wv, wp)
    nc.compile()
    _NC_CACHE["nc"] = nc
    return nc


def _prepare_in_maps(x, W_qkv, W_proj):
    x = np.asarray(x, dtype=np.float32)
    W_qkv = np.asarray(W_qkv, dtype=np.float32)
    W_proj = np.asarray(W_proj, dtype=np.float32)
    in_maps = []
    for c in range(N_CORES):
        b, hg = divmod(c, 2)
        cs = slice(hg * 512, (hg + 1) * 512)
        xTc = np.ascontiguousarray(x[b].T).astype(BF16)
        wqk = np.ascontiguousarray(
            np.concatenate([W_qkv[:, 0:1024][:, cs], W_qkv[:, 1024:2048][:, cs]], axis=1)
        ).astype(BF16)
        wv = np.ascontiguousarray(W_qkv[:, 2048:3072][:, cs]).astype(BF16)
        wp = np.ascontiguousarray(W_proj[cs, :]).astype(BF16)
        in_maps.append({"xT": xTc, "wqk": wqk, "wv": wv, "wp": wp})
    return in_maps


def _run(x, W_qkv, W_proj, b_proj, trace=False):
    nc = _build_nc()
    in_maps = _prepare_in_maps(x, W_qkv, W_proj)
    res = bass_utils.run_bass_kernel_spmd(
        nc, in_maps, core_ids=list(range(N_CORES)), trace=trace
    )
    b_proj = np.asarray(b_proj, dtype=np.float32)
    y = np.empty((4, NT, D), dtype=np.float32)
    for b in range(4):
        y[b] = res.results[2 * b]["y"] + res.results[2 * b + 1]["y"] + b_proj[None, :]
    return y, res


def kernel(x, W_qkv, W_proj, b_proj):
    y, _ = _run(x, W_qkv, W_proj, b_proj, trace=False)
    return y


# revision 19
# speedup vs baseline: 1.2107x; 1.2107x over previous
"""Trainium2 Bass kernel for nn_Attention (B=4, N=2048, D=1024, H=16, Hd=64).

Sharding: 8 cores = 4 batches x 2 head-groups. Core c handles batch c//2 and
heads [ (c%2)*8, (c%2)*8+8 ).  Each core computes qkv projections for its
heads, attention, and a partial output projection (contraction over its 512
head-dims of W_proj). Host sums the two partials per batch and adds b_proj.

Per-core kernel (all matmuls bf16 with fp32 PSUM accumulation):
  - qkT[f, t]  = sum_d Wqk[d, f] * xT[d, t]     (Q^T/K^T per head, [64, 2048])
  - v[t, f]    = sum_d xT[d, t] * Wv[d, f]       ([2048, 512], keys-major)
  - per head pair (2 heads packed in PE row/col groups):
      S^T[k, q] = sum_d K^T[d, k] Q^T[d, q]      (keys on partitions)
      E = exp(S^T / 8)   (ScalarE, bf16 out)
      U^T[hd, q] += sum_k V[k, hd] E[k, q]       (PSUM accumulate over key tiles)
      Eacc += E (VectorE);  sums = partition_all_reduce(Eacc)  (GpSimd)
      Uhat = U^T * (1/sums)                      (normalize during PSUM drain)
  - y[q, e] = sum_hd Uhat[hd, q] Wp[hd, e]       (partial; host adds pair+bias)
"""

import os
import sys
import types

import numpy as np

# --- environment bootstrap (grading env == dev env: axon-tunneled trn2) ----
for _p in ("/opt/trn_rl_repo", "/root/.axon_site/_ro/trn_rl_repo"):
    if _p not in sys.path and os.path.isdir(_p):
        sys.path.append(_p)

import ml_dtypes  # noqa: E402

BF16 = ml_dtypes.bfloat16


def _install_ntff_shim():
    """antenv.axon_hooks is missing on this image; provide it and register the
    ctypes NTFF hook so trace=True can report HW exec time."""
    if "antenv.axon_hooks" in sys.modules:
        return
    mod = types.ModuleType("antenv.axon_hooks")
    mod._hook = None
    mod.set_axon_ntff_profile_hook = lambda h: setattr(mod, "_hook", h)
    mod.get_axon_ntff_profile_hook = lambda: mod._hook
    sys.modules["antenv.axon_hooks"] = mod
    try:
        import antenv

        antenv.axon_hooks = mod
    except ImportError:
        pass
    try:
        from trn_agent_boot.trn_boot import _ntff_profile_via_ctypes

        hook = _ntff_profile_via_ctypes("/opt/axon/libaxon_pjrt.so")
        if hook is not None:
            mod.set_axon_ntff_profile_hook(hook)
    except Exception:
        pass


_install_ntff_shim()

import concourse.bacc as bacc  # noqa: E402
import concourse.bass as bass  # noqa: E402
import concourse.tile as tile  # noqa: E402
import concourse.bass_isa as bass_isa  # noqa: E402
from concourse import mybir  # noqa: E402
import concourse.bass_utils as bass_utils  # noqa: E402

# no S3 in the container; keep NTFF artifacts local
bass_utils.upload_artifacts = lambda tmpdir: tmpdir

F32 = mybir.dt.float32
BF = mybir.dt.bfloat16
EXP = mybir.ActivationFunctionType.Exp

N_CORES = 8
NT = 2048  # tokens
D = 1024  # d_model
NH_LOC = 8  # heads per core
HD = 64  # head dim
SCALE = HD**-0.5


def _body(tc: "tile.TileContext", ctx, y, xT, wqk, wv, wp):
    nc = tc.nc

    wpool = ctx.enter_context(tc.tile_pool(name="wpool", bufs=1))
    qkpool = ctx.enter_context(tc.tile_pool(name="qkpool", bufs=1))
    vpool = ctx.enter_context(tc.tile_pool(name="vpool", bufs=1))
    upool = ctx.enter_context(tc.tile_pool(name="upool", bufs=1))
    epool = ctx.enter_context(tc.tile_pool(name="epool", bufs=5))
    eaccpool = ctx.enter_context(tc.tile_pool(name="eaccpool", bufs=3))
    spool = ctx.enter_context(tc.tile_pool(name="spool", bufs=1))
    rpool = ctx.enter_context(tc.tile_pool(name="rpool", bufs=1))
    opool = ctx.enter_context(tc.tile_pool(name="opool", bufs=2))
    psb = ctx.enter_context(tc.tile_pool(name="psb", bufs=2, space="PSUM"))
    psu = ctx.enter_context(tc.tile_pool(name="psu", bufs=1, space="PSUM"))

    # ---- persistent SBUF tensors -----------------------------------------
    xT_sb = []
    for i in range(8):
        t = wpool.tile([128, NT], BF, tag=f"xT{i}", name=f"xT{i}")
        nc.sync.dma_start(out=t, in_=xT[i * 128 : (i + 1) * 128, :])
        xT_sb.append(t)
    wqk_sb = []
    for i in range(8):
        t = wpool.tile([128, 1024], BF, tag=f"wqk{i}", name=f"wqk{i}")
        nc.sync.dma_start(out=t, in_=wqk[i * 128 : (i + 1) * 128, :])
        wqk_sb.append(t)
    wv_sb = []
    for i in range(8):
        t = wpool.tile([128, 512], BF, tag=f"wv{i}", name=f"wv{i}")
        nc.sync.dma_start(out=t, in_=wv[i * 128 : (i + 1) * 128, :])
        wv_sb.append(t)
    wp_sb = []
    for i in range(4):
        t = wpool.tile([128, 1024], BF, tag=f"wp{i}", name=f"wp{i}")
        nc.sync.dma_start(out=t, in_=wp[i * 128 : (i + 1) * 128, :])
        wp_sb.append(t)

    qkT = [qkpool.tile([128, NT], BF, tag=f"qkT{f}", name=f"qkT{f}") for f in range(8)]
    v_sb = [vpool.tile([128, 512], BF, tag=f"v{t}", name=f"v{t}") for t in range(16)]
    uhat = [upool.tile([128, NT], BF, tag=f"uh{p}", name=f"uh{p}") for p in range(4)]
    ones64 = wpool.tile([128, 64], BF, tag="ones64", name="ones64")
    nc.vector.memset(ones64, 1.0)

    # ---- qkv projections --------------------------------------------------
    def qk_unit(f):
        # qkT[f][ff, t] = sum_d wqk[d, f*128+ff] * xT[d, t]
        for ts in range(2):
            ps = psb.tile([128, 1024], F32, tag="psb", name=f"qk_ps{f}_{ts}")
            for d in range(8):
                for s in range(2):
                    nc.tensor.matmul(
                        ps[:, s * 512 : (s + 1) * 512],
                        wqk_sb[d][:, f * 128 : (f + 1) * 128],
                        xT_sb[d][:, ts * 1024 + s * 512 : ts * 1024 + (s + 1) * 512],
                        start=(d == 0),
                        stop=(d == 7),
                    )
            nc.vector.tensor_copy(out=qkT[f][:, ts * 1024 : (ts + 1) * 1024], in_=ps[:])

    def v_unit(t):
        # v[t*128+tt, f] = sum_d xT[d, t*128+tt] * wv[d, f]; psum shares psu pool
        ps = psu.tile([128, 512], F32, tag="ut", name=f"v_ps{t}")
        for d in range(8):
            nc.tensor.matmul(
                ps[:, :],
                xT_sb[d][:, t * 128 : (t + 1) * 128],
                wv_sb[d][:, :],
                start=(d == 0),
                stop=(d == 7),
            )
        nc.vector.tensor_copy(out=v_sb[t], in_=ps[:])

    # ---- attention for one pair of heads (A=2p at rows 0:64, B at 64:128) -
    def attention_pair(p):
        A, B = 2 * p, 2 * p + 1
        qA = qkT[p][0:64, :]
        qB = qkT[p][64:128, :]
        kA = qkT[4 + p][0:64, :]
        kB = qkT[4 + p][64:128, :]
        ut = psu.tile([128, NT], F32, tag="ut", name=f"ut{p}")
        eaccA = eaccpool.tile([128, NT], BF, tag="eacc", name=f"eaccA{p}")
        eaccB = eaccpool.tile([128, NT], BF, tag="eacc", name=f"eaccB{p}")
        # Software-pipelined kt loop: the PV matmuls for key-tile kt-1 (inputs
        # long ready) are emitted between the QK score groups so the in-order
        # PE queue never stalls on the ACT exp stagger.
        eAs, eBs = [], []

        def pv(ktpv, half, s):
            e, head, r0 = (eAs[ktpv], A, 0) if half == 0 else (eBs[ktpv], B, 64)
            ssl = slice(s * 512, (s + 1) * 512)
            nc.tensor.matmul(
                ut[r0 : r0 + 64, ssl],
                v_sb[ktpv][:, head * 64 : (head + 1) * 64],
                e[:, ssl],
                start=(ktpv == 0),
                stop=(ktpv == 15),
            )

        def eacc_step(ktpv):
            if ktpv == 0:
                nc.vector.tensor_copy(out=eaccA, in_=eAs[0])
                nc.vector.tensor_copy(out=eaccB, in_=eBs[0])
            else:
                nc.vector.tensor_add(out=eaccA, in0=eaccA, in1=eAs[ktpv])
                nc.vector.tensor_add(out=eaccB, in0=eaccB, in1=eBs[ktpv])

        for kt in range(16):
            ksl = slice(kt * 128, (kt + 1) * 128)
            eA = epool.tile([128, NT], BF, tag="e", name=f"eA{p}_{kt}")
            eB = epool.tile([128, NT], BF, tag="e", name=f"eB{p}_{kt}")
            eAs.append(eA)
            eBs.append(eB)
            for qh in range(2):
                qsl = slice(qh * 1024, (qh + 1) * 1024)
                stA = psb.tile([128, 1024], F32, tag="psb", name=f"stA{p}_{kt}_{qh}")
                for s in range(2):
                    q0 = qh * 1024 + s * 512
                    nc.tensor.matmul(
                        stA[:, s * 512 : (s + 1) * 512], kA[:, ksl], qA[:, q0 : q0 + 512],
                        start=True, stop=True,
                    )
                nc.scalar.activation(out=eA[:, qsl], in_=stA[:], func=EXP, scale=SCALE)
                if kt > 0:
                    pv(kt - 1, 0, 2 * qh)
                    pv(kt - 1, 0, 2 * qh + 1)
                stB = psb.tile([128, 1024], F32, tag="psb", name=f"stB{p}_{kt}_{qh}")
                for s in range(2):
                    q0 = qh * 1024 + s * 512
                    nc.tensor.matmul(
                        stB[:, s * 512 : (s + 1) * 512], kB[:, ksl], qB[:, q0 : q0 + 512],
                        start=True, stop=True,
                    )
                nc.scalar.activation(out=eB[:, qsl], in_=stB[:], func=EXP, scale=SCALE)
                if kt > 0:
                    pv(kt - 1, 1, 2 * qh)
                    pv(kt - 1, 1, 2 * qh + 1)
            if kt > 0:
                eacc_step(kt - 1)
        # drain the pipelined tail (kt = 15)
        for s in range(4):
            pv(15, 0, s)
            pv(15, 1, s)
        eacc_step(15)
        # drain U^T (unnormalized) to SBUF right away so the PSUM accumulator
        # frees for the next pair; normalization happens off the critical path.
        for qh in range(2):
            qsl = slice(qh * 1024, (qh + 1) * 1024)
            nc.vector.tensor_copy(out=uhat[p][:, qsl], in_=ut[:, qsl])
        # softmax denominators: sums over the 128 key partitions via a
        # ones-matmul (PE, broadcast into rows; A -> 0:64, B -> 64:128) into
        # psb tiles (so they don't serialize behind the U^T drain), then
        # reciprocal on a partition-spread [128, 16] view (DVE divide is
        # ~6 cycles/elem, so never run it on a 2048-wide row), then broadcast
        # the reciprocal row to all partitions (GpSimd partition_broadcast).
        sums_ps = psu.tile([128, NT], F32, tag="ut", name=f"sums_ps{p}")
        for s in range(4):
            ssl = slice(s * 512, (s + 1) * 512)
            nc.tensor.matmul(sums_ps[0:64, ssl], ones64[:], eaccA[:, ssl], start=True, stop=True)
            nc.tensor.matmul(sums_ps[64:128, ssl], ones64[:], eaccB[:, ssl], start=True, stop=True)
        sums = spool.tile([128, NT], F32, tag="sums", name=f"sums{p}")
        nc.vector.tensor_copy(out=sums, in_=sums_ps[:])
        for half in (0, 1):
            r0 = half * 64
            rsp = spool.tile([128, 16], F32, tag="rsp", name=f"rsp{p}_{half}")
            row = sums[r0 : r0 + 1, :].rearrange("p (a b) -> p a b", a=128)
            nc.gpsimd.dma_start(out=rsp[:], in_=row)
            rspr = spool.tile([128, 16], F32, tag="rspr", name=f"rspr{p}_{half}")
            nc.vector.reciprocal(out=rspr[:], in_=rsp[:])
            rrow = spool.tile([1, NT], F32, tag="rrow", bufs=1, name=f"rrow{p}_{half}")
            nc.gpsimd.dma_start(
                out=rrow[0:1, :].rearrange("p (a b) -> p a b", a=128), in_=rspr[:]
            )
            rec = rpool.tile([128, NT], F32, tag=f"rec{half}", name=f"rec{p}_{half}")
            nc.gpsimd.partition_broadcast(out_ap=rec[:, :], in_ap=rrow[0:1, :])
            # normalize this half in place (bf16 * f32 -> bf16)
            for qh in range(2):
                qsl = slice(qh * 1024, (qh + 1) * 1024)
                nc.vector.tensor_mul(
                    uhat[p][r0 : r0 + 64, qsl], uhat[p][r0 : r0 + 64, qsl], rec[r0 : r0 + 64, qsl]
                )

    # ---- output projection (partial over local 512 head dims) ------------
    def proj_unit(qt):
        pj = psb.tile([128, 1024], F32, tag="psb", name=f"pj{qt}")
        for es in range(2):
            for c in range(4):
                nc.tensor.matmul(
                    pj[:, es * 512 : (es + 1) * 512],
                    uhat[c][:, qt * 128 : (qt + 1) * 128],
                    wp_sb[c][:, es * 512 : (es + 1) * 512],
                    start=(c == 0),
                    stop=(c == 3),
                )
        ot = opool.tile([128, 1024], F32, tag="out", name=f"ot{qt}")
        nc.vector.tensor_copy(out=ot, in_=pj[:])
        nc.sync.dma_start(out=y[qt * 128 : (qt + 1) * 128, :], in_=ot)

    # ---- schedule ---------------------------------------------------------
    qk_unit(0)
    qk_unit(4)
    for t in range(16):
        v_unit(t)
    qk_unit(1)
    qk_unit(5)
    qk_unit(2)
    qk_unit(6)
    qk_unit(3)
    qk_unit(7)
    for p in range(4):
        attention_pair(p)
    for qt in range(16):
        proj_unit(qt)


_NC_CACHE = {}


def _build_nc():
    if "nc" in _NC_CACHE:
        return _NC_CACHE["nc"]
    nc = bacc.Bacc("TRN2", target_bir_lowering=False, debug=False, num_devices=N_CORES)
    xT = nc.dram_tensor("xT", [D, NT], BF, kind="ExternalInput").ap()
    wqk = nc.dram_tensor("wqk", [D, 1024], BF, kind="ExternalInput").ap()
    wv = nc.dram_tensor("wv", [D, 512], BF, kind="ExternalInput").ap()
    wp = nc.dram_tensor("wp", [512, 1024], BF, kind="ExternalInput").ap()
    y = nc.dram_tensor("y", [NT, 1024], F32, kind="ExternalOutput").ap()
    from contextlib import ExitStack

    with tile.TileContext(nc) as tc, ExitStack() as ctx:
        _body(tc, ctx, y, xT, wqk, wv, wp)
    nc.compile()
    _NC_CACHE["nc"] = nc
    return nc


def _prepare_in_maps(x, W_qkv, W_proj):
    x = np.asarray(x, dtype=np.float32)
    W_qkv = np.asarray(W_qkv, dtype=np.float32)
    W_proj = np.asarray(W_proj, dtype=np.float32)
    in_maps = []
    for c in range(N_CORES):
        b, hg = divmod(c, 2)
        cs = slice(hg * 512, (hg + 1) * 512)
        xTc = np.ascontiguousarray(x[b].T).astype(BF16)
        wqk = np.ascontiguousarray(
            np.concatenate([W_qkv[:, 0:1024][:, cs], W_qkv[:, 1024:2048][:, cs]], axis=1)
        ).astype(BF16)
        wv = np.ascontiguousarray(W_qkv[:, 2048:3072][:, cs]).astype(BF16)
        wp = np.ascontiguousarray(W_proj[cs, :]).astype(BF16)
        in_maps.append({"xT": xTc, "wqk": wqk, "wv": wv, "wp": wp})
    return in_maps


def _run(x, W_qkv, W_proj, b_proj, trace=False):
    nc = _build_nc()
    in_maps = _prepare_in_maps(x, W_qkv, W_proj)
    res = bass_utils.run_bass_kernel_spmd(
        nc, in_maps, core_ids=list(range(N_CORES)), trace=trace
    )
    b_proj = np.asarray(b_proj, dtype=np.float32)
    y = np.empty((4, NT, D), dtype=np.float32)
    for b in range(4):
        y[b] = res.results[2 * b]["y"] + res.results[2 * b + 1]["y"] + b_proj[None, :]
    return y, res


def kernel(x, W_qkv, W_proj, b_proj):
    y, _ = _run(x, W_qkv, W_proj, b_proj, trace=False)
    return y


# revision 23
# speedup vs baseline: 1.4810x; 1.2233x over previous
"""Trainium2 Bass kernel for nn_Attention (B=4, N=2048, D=1024, H=16, Hd=64).

Sharding: 8 cores = 4 batches x 2 head-groups. Core c handles batch c//2 and
heads [ (c%2)*8, (c%2)*8+8 ).  Each core computes qkv projections for its
heads, attention, and a partial output projection (contraction over its 512
head-dims of W_proj). Host sums the two partials per batch and adds b_proj.

Per-core kernel (all matmuls bf16 with fp32 PSUM accumulation):
  - qkT[f, t]  = sum_d Wqk[d, f] * xT[d, t]     (Q^T/K^T per head, [64, 2048])
  - v[t, f]    = sum_d xT[d, t] * Wv[d, f]       ([2048, 512], keys-major)
  - per head pair (2 heads packed in PE row/col groups):
      S^T[k, q] = sum_d K^T[d, k] Q^T[d, q]      (keys on partitions)
      E = exp(S^T / 8)   (ScalarE, bf16 out)
      U^T[hd, q] += sum_k V[k, hd] E[k, q]       (PSUM accumulate over key tiles)
      Eacc += E (VectorE);  sums = partition_all_reduce(Eacc)  (GpSimd)
      Uhat = U^T * (1/sums)                      (normalize during PSUM drain)
  - y[q, e] = sum_hd Uhat[hd, q] Wp[hd, e]       (partial; host adds pair+bias)
"""

import os
import sys
import types

import numpy as np

# --- environment bootstrap (grading env == dev env: axon-tunneled trn2) ----
for _p in ("/opt/trn_rl_repo", "/root/.axon_site/_ro/trn_rl_repo"):
    if _p not in sys.path and os.path.isdir(_p):
        sys.path.append(_p)

import ml_dtypes  # noqa: E402

BF16 = ml_dtypes.bfloat16


def _install_ntff_shim():
    """antenv.axon_hooks is missing on this image; provide it and register the
    ctypes NTFF hook so trace=True can report HW exec time."""
    if "antenv.axon_hooks" in sys.modules:
        return
    mod = types.ModuleType("antenv.axon_hooks")
    mod._hook = None
    mod.set_axon_ntff_profile_hook = lambda h: setattr(mod, "_hook", h)
    mod.get_axon_ntff_profile_hook = lambda: mod._hook
    sys.modules["antenv.axon_hooks"] = mod
    try:
        import antenv

        antenv.axon_hooks = mod
    except ImportError:
        pass
    try:
        from trn_agent_boot.trn_boot import _ntff_profile_via_ctypes

        hook = _ntff_profile_via_ctypes("/opt/axon/libaxon_pjrt.so")
        if hook is not None:
            mod.set_axon_ntff_profile_hook(hook)
    except Exception:
        pass


_install_ntff_shim()

import concourse.bacc as bacc  # noqa: E402
import concourse.bass as bass  # noqa: E402
import concourse.tile as tile  # noqa: E402
import concourse.bass_isa as bass_isa  # noqa: E402
from concourse import mybir  # noqa: E402
import concourse.bass_utils as bass_utils  # noqa: E402

# no S3 in the container; keep NTFF artifacts local
bass_utils.upload_artifacts = lambda tmpdir: tmpdir

F32 = mybir.dt.float32
BF = mybir.dt.bfloat16
EXP = mybir.ActivationFunctionType.Exp

N_CORES = 8
NT = 2048  # tokens
D = 1024  # d_model
NH_LOC = 8  # heads per core
HD = 64  # head dim
SCALE = HD**-0.5


def _body(tc: "tile.TileContext", ctx, y, xT, wqk, wv, wp):
    nc = tc.nc

    wpool = ctx.enter_context(tc.tile_pool(name="wpool", bufs=1))
    qkpool = ctx.enter_context(tc.tile_pool(name="qkpool", bufs=1))
    vpool = ctx.enter_context(tc.tile_pool(name="vpool", bufs=1))
    upool = ctx.enter_context(tc.tile_pool(name="upool", bufs=1))
    epool = ctx.enter_context(tc.tile_pool(name="epool", bufs=8))
    eaccpool = ctx.enter_context(tc.tile_pool(name="eaccpool", bufs=3))
    spool = ctx.enter_context(tc.tile_pool(name="spool", bufs=1))
    rpool = ctx.enter_context(tc.tile_pool(name="rpool", bufs=1))
    opool = ctx.enter_context(tc.tile_pool(name="opool", bufs=3))
    # PSUM budget (8 banks): scores 2x[128,1024] (4) + U^T/sums [128,1024]
    # (2) + filler pool 2x[128,512] (2). The filler pool decouples qkv/proj
    # background matmuls from the score/exp pipeline slots.
    psb = ctx.enter_context(tc.tile_pool(name="psb", bufs=2, space="PSUM"))
    psu = ctx.enter_context(tc.tile_pool(name="psu", bufs=1, space="PSUM"))
    pfill = ctx.enter_context(tc.tile_pool(name="pfill", bufs=2, space="PSUM"))

    # ---- persistent SBUF tensors -----------------------------------------
    xT_sb = []
    for i in range(8):
        t = wpool.tile([128, NT], BF, tag=f"xT{i}", name=f"xT{i}")
        nc.sync.dma_start(out=t, in_=xT[i * 128 : (i + 1) * 128, :])
        xT_sb.append(t)
    wqk_sb = []
    for i in range(8):
        t = wpool.tile([128, 1024], BF, tag=f"wqk{i}", name=f"wqk{i}")
        nc.sync.dma_start(out=t, in_=wqk[i * 128 : (i + 1) * 128, :])
        wqk_sb.append(t)
    wv_sb = []
    for i in range(8):
        t = wpool.tile([128, 512], BF, tag=f"wv{i}", name=f"wv{i}")
        nc.sync.dma_start(out=t, in_=wv[i * 128 : (i + 1) * 128, :])
        wv_sb.append(t)
    wp_sb = []
    for i in range(4):
        t = wpool.tile([128, 1024], BF, tag=f"wp{i}", name=f"wp{i}")
        nc.sync.dma_start(out=t, in_=wp[i * 128 : (i + 1) * 128, :])
        wp_sb.append(t)

    qkT = [qkpool.tile([128, NT], BF, tag=f"qkT{f}", name=f"qkT{f}") for f in range(8)]
    v_sb = [vpool.tile([128, 512], BF, tag=f"v{t}", name=f"v{t}") for t in range(16)]
    uhat = [upool.tile([128, NT], BF, tag=f"uh{p}", name=f"uh{p}") for p in range(4)]
    ones64 = wpool.tile([128, 64], BF, tag="ones64", name="ones64")
    nc.vector.memset(ones64, 1.0)

    # ---- background units (run on the filler PSUM pool) -------------------
    def qk_sub(f, ts2):
        # qkT[f][:, ts2*512:(ts2+1)*512] = (x @ Wqk[:, f-chunk]).T slice
        ps = pfill.tile([128, 512], F32, tag="pf", name=f"qk_ps{f}_{ts2}")
        for d in range(8):
            nc.tensor.matmul(
                ps[:, :],
                wqk_sb[d][:, f * 128 : (f + 1) * 128],
                xT_sb[d][:, ts2 * 512 : (ts2 + 1) * 512],
                start=(d == 0),
                stop=(d == 7),
            )
        nc.vector.tensor_copy(out=qkT[f][:, ts2 * 512 : (ts2 + 1) * 512], in_=ps[:])

    def v_unit(t):
        ps = pfill.tile([128, 512], F32, tag="pf", name=f"v_ps{t}")
        for d in range(8):
            nc.tensor.matmul(
                ps[:, :],
                xT_sb[d][:, t * 128 : (t + 1) * 128],
                wv_sb[d][:, :],
                start=(d == 0),
                stop=(d == 7),
            )
        nc.vector.tensor_copy(out=v_sb[t], in_=ps[:])

    def proj_sub(qt, es):
        # y[qt-tile, es-slice] partial over this core's 512 head dims
        pj = pfill.tile([128, 512], F32, tag="pf", name=f"pj{qt}_{es}")
        for c in range(4):
            nc.tensor.matmul(
                pj[:, :],
                uhat[c][:, qt * 128 : (qt + 1) * 128],
                wp_sb[c][:, es * 512 : (es + 1) * 512],
                start=(c == 0),
                stop=(c == 3),
            )
        ot = opool.tile([128, 512], F32, tag="out", name=f"ot{qt}_{es}")
        nc.vector.tensor_copy(out=ot, in_=pj[:])
        nc.sync.dma_start(
            out=y[qt * 128 : (qt + 1) * 128, es * 512 : (es + 1) * 512], in_=ot
        )

    # ---- attention for one pair of heads, one query half ------------------
    # `fillers`: background units woven one-per-kt-step into this pair's
    # stream. Every filler MUST be emitted before the first instruction that
    # consumes its output (in-order engine queues deadlock otherwise), so
    # each list is fully drained inside its own pair-half (15 slots >= len).
    def attention_pair_half(p, half, fillers=()):
        fillers = list(fillers)
        assert len(fillers) <= 15
        A, B = 2 * p, 2 * p + 1
        hsl = slice(half * 1024, (half + 1) * 1024)
        qA = qkT[p][0:64, hsl]
        qB = qkT[p][64:128, hsl]
        kA = qkT[4 + p][0:64, :]
        kB = qkT[4 + p][64:128, :]
        ut = psu.tile([128, 1024], F32, tag="ut", name=f"ut{p}_{half}")
        eaccA = eaccpool.tile([128, 1024], BF, tag="eacc", name=f"eaccA{p}_{half}")
        eaccB = eaccpool.tile([128, 1024], BF, tag="eacc", name=f"eaccB{p}_{half}")
        eAs, eBs = [], []

        def pv(ktpv, hb, s):
            e, head, r0 = (eAs[ktpv], A, 0) if hb == 0 else (eBs[ktpv], B, 64)
            ssl = slice(s * 512, (s + 1) * 512)
            nc.tensor.matmul(
                ut[r0 : r0 + 64, ssl],
                v_sb[ktpv][:, head * 64 : (head + 1) * 64],
                e[:, ssl],
                start=(ktpv == 0),
                stop=(ktpv == 15),
            )

        def eacc_step(ktpv):
            if ktpv == 0:
                nc.vector.tensor_copy(out=eaccA, in_=eAs[0])
                nc.vector.tensor_copy(out=eaccB, in_=eBs[0])
            else:
                nc.vector.tensor_add(out=eaccA, in0=eaccA, in1=eAs[ktpv])
                nc.vector.tensor_add(out=eaccB, in0=eaccB, in1=eBs[ktpv])

        for kt in range(16):
            ksl = slice(kt * 128, (kt + 1) * 128)
            eA = epool.tile([128, 1024], BF, tag="e", name=f"eA{p}_{half}_{kt}")
            eB = epool.tile([128, 1024], BF, tag="e", name=f"eB{p}_{half}_{kt}")
            eAs.append(eA)
            eBs.append(eB)
            stA = psb.tile([128, 1024], F32, tag="psb", name=f"stA{p}_{half}_{kt}")
            for s in range(2):
                q0 = half * 1024 + s * 512
                nc.tensor.matmul(
                    stA[:, s * 512 : (s + 1) * 512], kA[:, ksl], qkT[p][0:64, q0 : q0 + 512],
                    start=True, stop=True,
                )
            nc.scalar.activation(out=eA[:], in_=stA[:], func=EXP, scale=SCALE)
            if kt > 0:
                pv(kt - 1, 0, 0)
                pv(kt - 1, 0, 1)
            stB = psb.tile([128, 1024], F32, tag="psb", name=f"stB{p}_{half}_{kt}")
            for s in range(2):
                q0 = half * 1024 + s * 512
                nc.tensor.matmul(
                    stB[:, s * 512 : (s + 1) * 512], kB[:, ksl], qkT[p][64:128, q0 : q0 + 512],
                    start=True, stop=True,
                )
            nc.scalar.activation(out=eB[:], in_=stB[:], func=EXP, scale=SCALE)
            if kt > 0:
                pv(kt - 1, 1, 0)
                pv(kt - 1, 1, 1)
                eacc_step(kt - 1)
            if kt > 0 and fillers:
                fillers.pop(0)()
        while fillers:
            fillers.pop(0)()
        for s in range(2):
            pv(15, 0, s)
            pv(15, 1, s)
        eacc_step(15)
        # drain U^T (unnormalized) so the PSUM accumulator frees quickly
        nc.vector.tensor_copy(out=uhat[p][:, hsl], in_=ut[:])
        # softmax denominators for this half (ones-matmul -> spread-recip ->
        # partition_broadcast), then normalize in place
        sums_ps = psu.tile([128, 1024], F32, tag="ut", name=f"sums_ps{p}_{half}")
        for s in range(2):
            ssl = slice(s * 512, (s + 1) * 512)
            nc.tensor.matmul(sums_ps[0:64, ssl], ones64[:], eaccA[:, ssl], start=True, stop=True)
            nc.tensor.matmul(sums_ps[64:128, ssl], ones64[:], eaccB[:, ssl], start=True, stop=True)
        sums = spool.tile([128, 1024], F32, tag="sums", name=f"sums{p}_{half}")
        nc.vector.tensor_copy(out=sums, in_=sums_ps[:])
        for hb in (0, 1):
            r0 = hb * 64
            rsp = spool.tile([128, 8], F32, tag="rsp", name=f"rsp{p}_{half}_{hb}")
            row = sums[r0 : r0 + 1, :].rearrange("p (a b) -> p a b", a=128)
            nc.gpsimd.dma_start(out=rsp[:], in_=row)
            rspr = spool.tile([128, 8], F32, tag="rspr", name=f"rspr{p}_{half}_{hb}")
            nc.vector.reciprocal(out=rspr[:], in_=rsp[:])
            rrow = spool.tile([1, 1024], F32, tag="rrow", bufs=1, name=f"rrow{p}_{half}_{hb}")
            nc.gpsimd.dma_start(
                out=rrow[0:1, :].rearrange("p (a b) -> p a b", a=128), in_=rspr[:]
            )
            rec = rpool.tile([128, 1024], F32, tag=f"rec{hb}", name=f"rec{p}_{half}_{hb}")
            nc.gpsimd.partition_broadcast(out_ap=rec[:, :], in_ap=rrow[0:1, :])
            nc.vector.tensor_mul(
                uhat[p][r0 : r0 + 64, hsl], uhat[p][r0 : r0 + 64, hsl], rec[r0 : r0 + 64, :]
            )

    # ---- schedule ---------------------------------------------------------
    # lead-in: q/k features for pair 0 plus the first v tiles; the rest of
    # the qkv projections and half-0's output projection weave into the
    # attention stream as per-pair filler lists (dependency-safe: each list
    # drains before the pair that consumes its outputs starts).
    def mk(fn, *args):
        return lambda: fn(*args)

    for ts2 in range(4):
        qk_sub(0, ts2)
        qk_sub(4, ts2)
    for t in range(10):
        v_unit(t)
    half0_fills = [
        [mk(v_unit, t) for t in range(10, 16)]
        + [mk(qk_sub, f, ts2) for f in (1, 5) for ts2 in range(4)],
        [mk(qk_sub, f, ts2) for f in (2, 6) for ts2 in range(4)],
        [mk(qk_sub, f, ts2) for f in (3, 7) for ts2 in range(4)],
        [],
    ]
    for p in range(4):
        attention_pair_half(p, 0, half0_fills[p])
    half1_fills = [
        [mk(proj_sub, qt, es) for qt in range(0, 4) for es in range(2)],
        [mk(proj_sub, qt, es) for qt in range(4, 8) for es in range(2)],
        [],
        [],
    ]
    for p in range(4):
        attention_pair_half(p, 1, half1_fills[p])
    for qt in range(8, 16):
        for es in range(2):
            proj_sub(qt, es)


_NC_CACHE = {}


def _build_nc():
    if "nc" in _NC_CACHE:
        return _NC_CACHE["nc"]
    nc = bacc.Bacc("TRN2", target_bir_lowering=False, debug=False, num_devices=N_CORES)
    xT = nc.dram_tensor("xT", [D, NT], BF, kind="ExternalInput").ap()
    wqk = nc.dram_tensor("wqk", [D, 1024], BF, kind="ExternalInput").ap()
    wv = nc.dram_tensor("wv", [D, 512], BF, kind="ExternalInput").ap()
    wp = nc.dram_tensor("wp", [512, 1024], BF, kind="ExternalInput").ap()
    y = nc.dram_tensor("y", [NT, 1024], F32, kind="ExternalOutput").ap()
    from contextlib import ExitStack

    with tile.TileContext(nc) as tc, ExitStack() as ctx:
        _body(tc, ctx, y, xT, wqk, wv, wp)
    nc.compile()
    _NC_CACHE["nc"] = nc
    return nc


def _prepare_in_maps(x, W_qkv, W_proj):
    x = np.asarray(x, dtype=np.float32)
    W_qkv = np.asarray(W_qkv, dtype=np.float32)
    W_proj = np.asarray(W_proj, dtype=np.float32)
    in_maps = []
    for c in range(N_CORES):
        b, hg = divmod(c, 2)
        cs = slice(hg * 512, (hg + 1) * 512)
        xTc = np.ascontiguousarray(x[b].T).astype(BF16)
        wqk = np.ascontiguousarray(
            np.concatenate([W_qkv[:, 0:1024][:, cs], W_qkv[:, 1024:2048][:, cs]], axis=1)
        ).astype(BF16)
        wv = np.ascontiguousarray(W_qkv[:, 2048:3072][:, cs]).astype(BF16)
        wp = np.ascontiguousarray(W_proj[cs, :]).astype(BF16)
        in_maps.append({"xT": xTc, "wqk": wqk, "wv": wv, "wp": wp})
    return in_maps


def _run(x, W_qkv, W_proj, b_proj, trace=False):
    nc = _build_nc()
    in_maps = _prepare_in_maps(x, W_qkv, W_proj)
    res = bass_utils.run_bass_kernel_spmd(
        nc, in_maps, core_ids=list(range(N_CORES)), trace=trace
    )
    b_proj = np.asarray(b_proj, dtype=np.float32)
    y = np.empty((4, NT, D), dtype=np.float32)
    for b in range(4):
        y[b] = res.results[2 * b]["y"] + res.results[2 * b + 1]["y"] + b_proj[None, :]
    return y, res


def kernel(x, W_qkv, W_proj, b_proj):
    y, _ = _run(x, W_qkv, W_proj, b_proj, trace=False)
    return y
